# revision 35
# baseline (speedup 1.0000x reference)
"""Tensor-parallel Trainium2 kernel for a single-token decode transformer block.

Sharding (8 cores, tensor-parallel over heads / FFN width):
  - Wq/Wk/Wv rows (head dim), KV caches over heads, attention per-core
  - Wo columns, partial out_proj -> AllReduce
  - W_up rows, W_down columns, partial down_proj -> AllReduce (b_down folded /8)
Inputs are full tensors; sharding/layout happens on the host. All arithmetic on device.
"""
import sys
from contextlib import ExitStack

sys.path.insert(0, "/opt/trn_rl_repo")
import numpy as np

import concourse.bacc as bacc
import concourse.tile as tile
import concourse.mybir as mybir
from concourse import bass_utils
from concourse.masks import make_identity

F32 = mybir.dt.float32
F32R = mybir.dt.float32r
AF = mybir.ActivationFunctionType
OP = mybir.AluOpType
EPS = 1e-5


class Cfg:
    def __init__(self, B=8, D=4096, H=32, DH=128, L=4096, DFF=16384, NC=8,
                 pv_f32r=True, ffn_f32r=True, bias_f32r=True, fake_cc=False):
        self.fake_cc = fake_cc
        self.B, self.D, self.H, self.DH, self.L, self.DFF, self.NC = B, D, H, DH, L, DFF, NC
        self.HL = H // NC            # heads per core
        self.E = self.HL * DH        # local attention width
        self.F = DFF // NC           # local ffn width
        self.ND = D // 128           # d 128-chunks
        self.NE = self.E // 128
        self.NF = self.F // 128
        self.NTL = L // 128          # l-tiles per (b,h)
        self.PAIRS = self.HL * B     # pair index = h*B + b
        self.KCH = min(2048, L)      # kT dma chunk cols
        self.VT = min(4, self.NTL)   # l-tiles per vP dma chunk
        self.WD = min(4, self.ND)    # d-chunks per qkv-weight dma chunk
        self.UD = 1                  # d-chunks per wupT dma chunk
        self.HALF = min(2048, D)     # psum-half width for [B, D] accumulators
        self.NH = D // self.HALF
        self.pv_f32r = pv_f32r
        self.ffn_f32r = ffn_f32r
        self.bias_f32r = bias_f32r
        assert B == 8 and DH == 128 and self.E <= 512 and self.F <= 2048


def build_nc(cfg: Cfg, num_devices=None):
    c = cfg
    B, D, L = c.B, c.D, c.L
    nc = bacc.Bacc("TRN2", target_bir_lowering=False, debug=False,
                   num_devices=num_devices or c.NC)

    dt_in = {}
    def din(name, shape):
        dt_in[name] = nc.dram_tensor(name, shape, F32, kind="ExternalInput")
        return dt_in[name]

    x_in = din("x_in", [B, D])
    kT = din("kT", [B, c.HL, c.DH, L])
    vP = din("vP", [B, L, c.E])
    wqT = din("wqT", [D, c.E])
    wkT = din("wkT", [D, c.E])
    wvT = din("wvT", [D, c.E])
    woT = din("woT", [c.E, D])
    wupT = din("wupT", [D, c.F])
    wdnT = din("wdnT", [c.F, D])
    g1d = din("g1d", [1, D])
    b1d = din("b1d", [1, D])
    g2d = din("g2d", [1, D])
    b2d = din("b2d", [1, D])
    bupd = din("bupd", [1, c.F])
    bdnd = din("bdnd", [1, D])
    out = nc.dram_tensor("out", [B, D], F32, kind="ExternalOutput")

    ar_space = "Shared" if c.NC > 4 else "Local"
    ar1_in = nc.dram_tensor("ar1_in", [B, D], F32, kind="Internal")
    ar1_out = nc.dram_tensor("ar1_out", [B, D], F32, kind="Internal", addr_space=ar_space)
    ar2_in = nc.dram_tensor("ar2_in", [B, D], F32, kind="Internal")
    ar2_out = nc.dram_tensor("ar2_out", [B, D], F32, kind="Internal", addr_space=ar_space)
    groups = [list(range(c.NC))]

    BIAS_DT = F32R if c.bias_f32r else F32
    PV_DT = F32R if c.pv_f32r else F32
    FFN_DT = F32R if c.ffn_f32r else F32

    with tile.TileContext(nc) as tc, ExitStack() as st:
        const_pool = st.enter_context(tc.tile_pool(name="const", bufs=1))
        sb_keep = st.enter_context(tc.tile_pool(name="sbk", bufs=1))
        row8_pool = st.enter_context(tc.tile_pool(name="row8", bufs=3))
        vec_pool = st.enter_context(tc.tile_pool(name="vec", bufs=1))
        kpool = st.enter_context(tc.tile_pool(name="kpool", bufs=3))
        vpool = st.enter_context(tc.tile_pool(name="vpool", bufs=3))
        wpool = st.enter_context(tc.tile_pool(name="wpool", bufs=4))
        sb_small = st.enter_context(tc.tile_pool(name="sbs", bufs=2))
        ps_small = st.enter_context(tc.tile_pool(name="pss", bufs=2, space="PSUM"))

        # ---- constants ----
        ident = const_pool.tile([128, 128], F32)
        make_identity(nc, ident[:])
        # f32r constants must be DMA-produced (memset can't emit f32r, and the
        # verifier requires f32r producers); bitcast-DMA from inline tensors.
        ones_d = nc.inline_tensor(np.ones((1, 128), np.float32), name="ones_c")
        eighth_d = nc.inline_tensor(np.full((1, B), 1.0 / c.NC, np.float32),
                                    name="eighth_c")

        def const_from(tile_, src_ap):
            if tile_.dtype == F32R:
                nc.sync.dma_start(tile_[:], src_ap.bitcast(F32R))
            else:
                nc.sync.dma_start(tile_[:], src_ap)

        ones_col = const_pool.tile([128, 1], PV_DT)
        const_from(ones_col, ones_d.ap().rearrange("o p -> p o"))
        ones_col32 = const_pool.tile([128, 1], F32)
        nc.gpsimd.memset(ones_col32[:], 1.0)
        ones_row = const_pool.tile([1, B], BIAS_DT)
        const_from(ones_row, ones_d.ap()[:, :B])
        eighth_row = const_pool.tile([1, B], BIAS_DT)
        const_from(eighth_row, eighth_d.ap())
        eps_col = const_pool.tile([B, 1], F32)
        nc.gpsimd.memset(eps_col[:], EPS)
        ones_row128 = const_pool.tile([1, 128], F32)
        nc.gpsimd.memset(ones_row128[:], 1.0)

        x_sb = sb_keep.tile([B, D], F32, tag="x_sb")
        nc.sync.dma_start(x_sb[:], x_in.ap())
        x1_sb = sb_keep.tile([B, D], F32, tag="x1_sb")

        def maybe_r(ap, dt):
            return ap.bitcast(dt) if dt == F32R and ap.dtype != F32R else ap

        def bcast_mms(ps_ap, vec_dram, col0, width, row_const, start, stop):
            vt = vec_pool.tile([1, c.HALF], BIAS_DT, tag="vec")
            nc.sync.dma_start(vt[:, :width],
                              maybe_r(vec_dram.ap()[:, col0:col0 + width], BIAS_DT))
            for i in range(0, width, 512):
                w = min(512, width - i)
                nc.tensor.matmul(ps_ap[:, i:i + w], row_const[:], vt[:, i:i + w],
                                 start=start, stop=stop)

        def layer_norm(x_ap, g_dram, b_dram, h_tile, wide_pool):
            """h_tile[:B, :D] = LN(x_ap) * g + b, natural [B, D] layout."""
            s1 = sb_small.tile([B, 1], F32, tag="lnstat")
            nc.vector.tensor_reduce(s1[:], x_ap, mybir.AxisListType.X, OP.add)
            mean = sb_small.tile([B, 1], F32, tag="lnstat")
            nc.vector.tensor_scalar(mean[:], s1[:], 1.0 / D, None, OP.mult)
            xc = row8_pool.tile([B, D], F32, tag="row8")
            nc.vector.tensor_scalar(xc[:], x_ap, mean[:], None, OP.subtract)
            ssq = sb_small.tile([B, 1], F32, tag="lnstat")
            nc.scalar.activation(h_tile[:], xc[:], AF.Square, accum_out=ssq[:])
            var = sb_small.tile([B, 1], F32, tag="lnstat")
            nc.vector.tensor_scalar(var[:], ssq[:], 1.0 / D, None, OP.mult)
            std = sb_small.tile([B, 1], F32, tag="lnstat")
            nc.scalar.activation(std[:], var[:], AF.Sqrt, bias=eps_col[:])
            rstd = sb_small.tile([B, 1], F32, tag="lnstat")
            nc.vector.reciprocal(rstd[:], std[:])
            hpre = row8_pool.tile([B, D], F32, tag="row8")
            nc.vector.tensor_scalar(hpre[:], xc[:], rstd[:], None, OP.mult)
            for j in range(c.NH):
                h0 = j * c.HALF
                g_ps = wide_pool.tile([B, c.HALF], F32, tag="psw")
                bcast_mms(g_ps[:], g_dram, h0, c.HALF, ones_row, True, True)
                nc.vector.tensor_tensor(h_tile[:, h0:h0 + c.HALF],
                                        hpre[:, h0:h0 + c.HALF], g_ps[:], OP.mult)
                b_ps = wide_pool.tile([B, c.HALF], F32, tag="psw")
                bcast_mms(b_ps[:], b_dram, h0, c.HALF, ones_row, True, True)
                nc.vector.tensor_tensor(h_tile[:, h0:h0 + c.HALF],
                                        h_tile[:, h0:h0 + c.HALF], b_ps[:], OP.add)

        def transpose_rows(src_ap, n_chunks, dst_tile):
            """src_ap [B, n_chunks*128] -> dst_tile [128, n_chunks*B]."""
            for s in range(0, n_chunks, 4):
                e = min(s + 4, n_chunks)
                tp = ps_small.tile([128, 4 * B], F32, tag="pst")
                for tt in range(s, e):
                    nc.tensor.transpose(tp[:, (tt - s) * B:(tt - s) * B + B],
                                        src_ap[:, tt * 128:tt * 128 + 128],
                                        ident[:B, :B])
                nc.any.tensor_copy(dst_tile[:, s * B:e * B], tp[:, :(e - s) * B])

        # ================= LN1 + h1^T =================
        h1 = row8_pool.tile([B, D], F32, tag="row8")
        with tc.tile_pool(name="psw1", bufs=1, space="PSUM") as wp:
            layer_norm(x_sb[:], g1d, b1d, h1, wp)
        h1T = sb_keep.tile([128, c.ND * B], F32, tag="h1T")
        transpose_rows(h1[:], c.ND, h1T)

        # ================= q/k/v projections =================
        qT = sb_keep.tile([128, c.NE * B], F32, tag="qT")
        kTn = sb_keep.tile([128, c.NE * B], F32, tag="kTn")
        vTn = sb_keep.tile([128, c.NE * B], F32, tag="vTn")
        with tc.tile_pool(name="psp", bufs=1, space="PSUM") as ps_proj:
            for pi, (wten, dst) in enumerate(((wqT, qT), (wkT, kTn), (wvT, vTn))):
                # one psum bank per e-chunk: accumulation groups must not share
                # a 2KB zero region
                ptiles = [ps_proj.tile([128, B], F32, tag=f"pj{ec}",
                                       name=f"pj{pi}_{ec}") for ec in range(c.NE)]
                for ch in range(c.ND // c.WD):
                    wt = wpool.tile([128, c.WD, c.E], F32, tag="w")
                    src = wten.ap().rearrange("(n p) e -> p n e", p=128)[
                        :, ch * c.WD:(ch + 1) * c.WD, :]
                    nc.sync.dma_start(wt[:], src)
                    for wd in range(c.WD):
                        dc = ch * c.WD + wd
                        for ec in range(c.NE):
                            nc.tensor.matmul(ptiles[ec][:],
                                             wt[:, wd, ec * 128:ec * 128 + 128],
                                             h1T[:, dc * B:dc * B + B],
                                             start=(dc == 0), stop=(dc == c.ND - 1))
                for ec in range(c.NE):
                    nc.any.tensor_copy(dst[:, ec * B:(ec + 1) * B], ptiles[ec][:])

        # ================= attention =================
        inv_sqrt_dh = 1.0 / float(np.sqrt(c.DH))
        exp_sc = sb_keep.tile([128, c.HL, B, c.NTL], PV_DT, tag="expsc")
        p_new = sb_small.tile([1, c.PAIRS], F32, tag="pnew")
        qk_el = sb_small.tile([128, c.PAIRS], F32, tag="qkel")
        with tc.tile_pool(name="pssc", bufs=1, space="PSUM") as ps_scores:
            # ---- QK ----
            scores_ps = ps_scores.tile([128, c.HL, B, c.NTL], F32)
            tiles_per_ch = c.KCH // 128
            for b in range(B):
                for h in range(c.HL):
                    for chk in range(L // c.KCH):
                        kt = kpool.tile([128, c.KCH], F32, tag="kc")
                        nc.sync.dma_start(
                            kt[:], kT.ap()[b, h, :, chk * c.KCH:(chk + 1) * c.KCH])
                        for i in range(tiles_per_ch):
                            lt = chk * tiles_per_ch + i
                            nc.tensor.matmul(scores_ps[:, h, b, lt:lt + 1],
                                             kt[:, i * 128:i * 128 + 128],
                                             qT[:, h * B + b:h * B + b + 1],
                                             start=True, stop=True)

            # ---- new-token score ----
            nc.vector.tensor_tensor(qk_el[:], qT[:], kTn[:], OP.mult)
            sn_ps = ps_small.tile([1, c.PAIRS], F32, tag="pst")
            nc.tensor.matmul(sn_ps[:], ones_col32[:], qk_el[:], start=True, stop=True)
            nc.scalar.activation(p_new[:], sn_ps[:], AF.Exp, scale=inv_sqrt_dh)

            # ---- softmax numerators (no max-sub; scores are O(10)) ----
            nc.scalar.activation(exp_sc[:], scores_ps[:], AF.Exp, scale=inv_sqrt_dh)
        exp_flat = exp_sc[:].rearrange("p h b l -> p (h b l)")

        with tc.tile_pool(name="psS", bufs=1, space="PSUM") as ps_S, \
             tc.tile_pool(name="pso", bufs=4, space="PSUM") as ps_oacc:
            # ---- denominators ----
            ncols = c.PAIRS * c.NTL
            S_ps = ps_S.tile([1, ncols], F32)
            for j in range(0, ncols, 512):
                w = min(512, ncols - j)
                nc.tensor.matmul(S_ps[:, j:j + w], ones_col[:], exp_flat[:, j:j + w],
                                 start=True, stop=True)
            S_red = sb_small.tile([1, c.PAIRS], F32, tag="sred")
            nc.vector.tensor_reduce(
                S_red[:], S_ps[:].rearrange("p (q l) -> p q l", l=c.NTL),
                mybir.AxisListType.X, OP.add)
            S_tot = sb_small.tile([1, c.PAIRS], F32, tag="stot")
            nc.vector.tensor_tensor(S_tot[:], S_red[:], p_new[:], OP.add)
            invS_row = sb_small.tile([1, c.PAIRS], F32, tag="invs")
            nc.vector.reciprocal(invS_row[:], S_tot[:])

            # ---- PV + per-b extraction ----
            # matmul dst must start at partition 0, so each b accumulates in
            # its own psum bank [HL, E]; extraction transposes [HL,128] chunks
            oT = sb_keep.tile([128, c.PAIRS], F32, tag="oT")
            oT_raw = sb_small.tile([128, c.PAIRS], F32, tag="otraw", bufs=1)
            for b in range(B):
                o_ps = ps_oacc.tile([c.HL, 512], F32, tag="oacc", name=f"oacc{b}")
                for ch in range(c.NTL // c.VT):
                    vt = vpool.tile([128, c.VT, c.E], PV_DT, tag="vc")
                    src = vP.ap().rearrange("b (n p) e -> b p n e", p=128)[
                        b, :, ch * c.VT:(ch + 1) * c.VT, :]
                    nc.sync.dma_start(vt[:], maybe_r(src, PV_DT))
                    for i in range(c.VT):
                        lt = ch * c.VT + i
                        nc.tensor.matmul(o_ps[:, :c.E], exp_sc[:, :, b, lt],
                                         vt[:, i, :],
                                         start=(lt == 0), stop=(lt == c.NTL - 1))
                o_sb = sb_small.tile([c.HL, 512], F32, tag="osb")
                nc.any.tensor_copy(o_sb[:, :c.E], o_ps[:, :c.E])
                for ec in range(c.NE):
                    tp = ps_small.tile([128, c.HL], F32, tag="pst")
                    nc.tensor.transpose(tp[:], o_sb[:, ec * 128:ec * 128 + 128],
                                        ident[:c.HL, :c.HL])
                    # tp[dh, h'] ; diagonal head of chunk ec is h'=ec
                    nc.any.tensor_copy(oT_raw[:, ec * B + b:ec * B + b + 1],
                                       tp[:, ec:ec + 1])
            # broadcast p_new and 1/S across partitions via contraction-1 matmul
            bc_ps = ps_small.tile([128, 2 * c.PAIRS], F32, tag="pst")
            nc.tensor.matmul(bc_ps[:, 0:c.PAIRS], ones_row128[:], p_new[:],
                             start=True, stop=True)
            nc.tensor.matmul(bc_ps[:, c.PAIRS:2 * c.PAIRS], ones_row128[:],
                             invS_row[:], start=True, stop=True)
            tmp = sb_small.tile([128, c.PAIRS], F32, tag="tmpc")
            nc.vector.tensor_tensor(tmp[:], vTn[:], bc_ps[:, 0:c.PAIRS], OP.mult)
            nc.vector.tensor_tensor(tmp[:], tmp[:], oT_raw[:], OP.add)
            nc.vector.tensor_tensor(oT[:], tmp[:], bc_ps[:, c.PAIRS:2 * c.PAIRS],
                                    OP.mult)

        # ================= out_proj =================
        # oT[dh, pair]; e-chunk h of o^T is oT[:, h*B:(h+1)*B]
        attn_sb = row8_pool.tile([B, D], F32, tag="row8")
        with tc.tile_pool(name="psw2", bufs=1, space="PSUM") as wp:
            for j in range(c.NH):
                a_ps = wp.tile([B, c.HALF], F32, tag="psw")
                for ec in range(c.NE):
                    wt = wpool.tile([128, c.HALF], F32, tag="w")
                    nc.sync.dma_start(wt[:], woT.ap()[ec * 128:ec * 128 + 128,
                                                      j * c.HALF:(j + 1) * c.HALF])
                    for i in range(c.HALF // 512):
                        nc.tensor.matmul(a_ps[:, i * 512:i * 512 + 512],
                                         oT[:, ec * B:ec * B + B],
                                         wt[:, i * 512:i * 512 + 512],
                                         start=(ec == 0), stop=(ec == c.NE - 1))
                nc.any.tensor_copy(attn_sb[:, j * c.HALF:(j + 1) * c.HALF], a_ps[:])

        # ================= AllReduce 1; x1 = x + sum =================
        nc.sync.dma_start(ar1_in.ap(), attn_sb[:])
        if c.fake_cc:
            nc.sync.dma_start(ar1_out.ap(), ar1_in.ap())
        else:
            nc.gpsimd.collective_compute("AllReduce", OP.add, replica_groups=groups,
                                         ins=[ar1_in.ap().opt()],
                                         outs=[ar1_out.ap().opt()])
        asum = row8_pool.tile([B, D], F32, tag="row8")
        nc.sync.dma_start(asum[:], ar1_out.ap())
        nc.vector.tensor_tensor(x1_sb[:], x_sb[:], asum[:], OP.add)

        # ================= LN2 + h2^T =================
        h2 = row8_pool.tile([B, D], F32, tag="row8")
        with tc.tile_pool(name="psw3", bufs=1, space="PSUM") as wp:
            layer_norm(x1_sb[:], g2d, b2d, h2, wp)
        h2T = sb_keep.tile([128, c.ND * B], FFN_DT, tag="h2T")
        transpose_rows(h2[:], c.ND, h2T)

        # ================= FFN up + relu =================
        ff = sb_small.tile([B, c.F], F32, tag="ff", bufs=1)
        with tc.tile_pool(name="psw4", bufs=1, space="PSUM") as wp:
            up_ps = wp.tile([B, c.F], F32, tag="psw")
            bcast_mms(up_ps[:], bupd, 0, c.F, ones_row, True, False)
            for ch in range(c.ND // c.UD):
                wt = wpool.tile([128, c.UD, c.F], FFN_DT, tag="w")
                src = wupT.ap().rearrange("(n p) f -> p n f", p=128)[
                    :, ch * c.UD:(ch + 1) * c.UD, :]
                nc.sync.dma_start(wt[:], maybe_r(src, FFN_DT))
                for ud in range(c.UD):
                    dc = ch * c.UD + ud
                    for i in range(c.F // 512):
                        nc.tensor.matmul(up_ps[:, i * 512:i * 512 + 512],
                                         h2T[:, dc * B:dc * B + B],
                                         wt[:, ud, i * 512:i * 512 + 512],
                                         start=False, stop=(dc == c.ND - 1))
            nc.scalar.activation(ff[:], up_ps[:], AF.Relu)
        ffT = sb_keep.tile([128, c.NF * B], FFN_DT, tag="ffT")
        transpose_rows(ff[:], c.NF, ffT)

        # ================= FFN down (+ bdn/NC) =================
        dn_sb = row8_pool.tile([B, D], F32, tag="row8")
        with tc.tile_pool(name="psw5", bufs=1, space="PSUM") as wp:
            for j in range(c.NH):
                d_ps = wp.tile([B, c.HALF], F32, tag="psw")
                bcast_mms(d_ps[:], bdnd, j * c.HALF, c.HALF, eighth_row, True, False)
                for fc in range(c.NF):
                    wt = wpool.tile([128, c.HALF], FFN_DT, tag="w")
                    src = wdnT.ap()[fc * 128:fc * 128 + 128,
                                    j * c.HALF:(j + 1) * c.HALF]
                    nc.sync.dma_start(wt[:], maybe_r(src, FFN_DT))
                    for i in range(c.HALF // 512):
                        nc.tensor.matmul(d_ps[:, i * 512:i * 512 + 512],
                                         ffT[:, fc * B:fc * B + B],
                                         wt[:, i * 512:i * 512 + 512],
                                         start=False, stop=(fc == c.NF - 1))
                nc.any.tensor_copy(dn_sb[:, j * c.HALF:(j + 1) * c.HALF], d_ps[:])

        # ================= AllReduce 2; out = x1 + sum =================
        nc.sync.dma_start(ar2_in.ap(), dn_sb[:])
        if c.fake_cc:
            nc.sync.dma_start(ar2_out.ap(), ar2_in.ap())
        else:
            nc.gpsimd.collective_compute("AllReduce", OP.add, replica_groups=groups,
                                         ins=[ar2_in.ap().opt()],
                                         outs=[ar2_out.ap().opt()])
        fsum = row8_pool.tile([B, D], F32, tag="row8")
        nc.sync.dma_start(fsum[:], ar2_out.ap())
        out_sb = row8_pool.tile([B, D], F32, tag="row8")
        nc.vector.tensor_tensor(out_sb[:], x1_sb[:], fsum[:], OP.add)
        nc.sync.dma_start(out.ap(), out_sb[:])

    nc.compile()
    return nc


def shard_inputs(cfg, x, key_cache, value_cache, Wq, Wk, Wv, Wo,
                 ln1_g, ln1_b, ln2_g, ln2_b, W_up, b_up, W_down, b_down):
    c = cfg
    asnp = lambda a: np.asarray(a, dtype=np.float32)
    x, key_cache, value_cache = asnp(x), asnp(key_cache), asnp(value_cache)
    Wq, Wk, Wv, Wo = asnp(Wq), asnp(Wk), asnp(Wv), asnp(Wo)
    W_up, W_down = asnp(W_up), asnp(W_down)
    vec = lambda a: asnp(a).reshape(1, -1)
    x2 = np.ascontiguousarray(x.reshape(c.B, c.D))
    in_maps = []
    for ci in range(c.NC):
        hs = slice(ci * c.HL, (ci + 1) * c.HL)
        es = slice(ci * c.E, (ci + 1) * c.E)
        fs = slice(ci * c.F, (ci + 1) * c.F)
        in_maps.append({
            "x_in": x2,
            "kT": np.ascontiguousarray(key_cache[:, hs].transpose(0, 1, 3, 2)),
            "vP": np.ascontiguousarray(
                value_cache[:, hs].transpose(0, 2, 1, 3).reshape(c.B, c.L, c.E)),
            "wqT": np.ascontiguousarray(Wq[es, :].T),
            "wkT": np.ascontiguousarray(Wk[es, :].T),
            "wvT": np.ascontiguousarray(Wv[es, :].T),
            "woT": np.ascontiguousarray(Wo[:, es].T),
            "wupT": np.ascontiguousarray(W_up[fs, :].T),
            "wdnT": np.ascontiguousarray(W_down[:, fs].T),
            "g1d": vec(ln1_g), "b1d": vec(ln1_b),
            "g2d": vec(ln2_g), "b2d": vec(ln2_b),
            "bupd": vec(b_up)[:, fs], "bdnd": vec(b_down),
        })
    return in_maps


_NC_CACHE = {}


def kernel(**inputs) -> np.ndarray:
    cfg = Cfg()
    if "full" not in _NC_CACHE:
        _NC_CACHE["full"] = build_nc(cfg)
    nc = _NC_CACHE["full"]
    in_maps = shard_inputs(cfg, **inputs)
    res = bass_utils.run_bass_kernel_spmd(nc, in_maps, core_ids=list(range(cfg.NC)))
    return np.asarray(res.results[0]["out"], dtype=np.float32).reshape(cfg.B, 1, cfg.D)
